# revision 2
# baseline (speedup 1.0000x reference)
"""PolyCntSketch (TensorSketch, degree 3) Trainium2 kernel.

Math: for each degree d, CountSketch_d = X @ S_d (S_d one-hot signed), then
out = irfft(prod_d rfft(CountSketch_d)).

Device strategy (pure data parallelism over batch, 8 cores):
  - Host feeds X transposed ([F, B_core]), features packed into 128-row chunks
    where each chunk holds whole (block_d0, block_d1, block_d2)-classes
    (block = idx_d // 128), so each chunk touches few 128-bucket blocks per
    degree -> few segment matmuls.
  - Stage 1: count sketch via per-(chunk, degree, block) matmuls
    (lhsT = signed one-hot Z [fill, 128]) accumulated in PSUM -> sketch [512, B].
  - Stage 2: rfft as DFT matmul (K = 512 buckets) -> Re/Im [256+Nyquist, B].
  - Stage 3: complex product across the 3 degrees on VectorE; the Nyquist bin
    (pure real) rides in the identically-zero Im(0) slot.
  - Stage 4: irfft as matmul -> out^T [512, B].
All matmuls run in float32r (fp32 rounded to 11-bit mantissa; full PE rate at
N >= 256). Host pre-rounds inputs so DMA can feed fp32r tiles directly.
"""
import sys

for _p in ("/opt/trn_rl_repo",):
    if _p not in sys.path:
        sys.path.append(_p)

import numpy as np

from concourse import bacc, mybir, tile
from concourse import bass_utils

F32R = mybir.dt.float32r
F32 = mybir.dt.float32

B, F, NCOMP, DEG = 8192, 4096, 512, 3
NCORES = 8
B_CORE = B // NCORES
BT = 256                     # batch columns per tile
CHUNK = 128
NBLK = NCOMP // 128          # 4 bucket blocks


def round_fp32r(x):
    b = np.ascontiguousarray(x, np.float32).view(np.uint32)
    t = b + 0x7FF + ((b >> 12) & 1)
    return (t & 0xFFFFF000).astype(np.uint32).view(np.float32)


def build_plan(index_hash, bit_hash):
    """FFD-pack whole (g0,g1,g2)-classes into 128-row chunks.

    Returns:
      order [F]: feature order for the transposed X upload
      chunks: list of (start, fill) row ranges into the ordered X
      plan[d][g]: list of (chunk_idx, zslot) in emission order ((d,g)-major)
      zm_t [128, nmm, 128]: stacked Z matrices, partition-major
    """
    idx = np.asarray(index_hash)
    sgn = (np.asarray(bit_hash) * 2 - 1).astype(np.float32)
    blocks = idx >> 7
    key = blocks[0] * 16 + blocks[1] * 4 + blocks[2]
    order_all = np.argsort(key, kind="stable")
    kvals = key[order_all]

    from collections import defaultdict
    sgroups = defaultdict(list)
    for kv in np.unique(kvals):
        f = order_all[kvals == kv]
        kv = int(kv)
        while len(f) > CHUNK:
            sgroups[kv >> 2].append((kv, f[:CHUNK]))
            f = f[CHUNK:]
        if len(f):
            sgroups[kv >> 2].append((kv, f))

    bins = []
    for sg in sorted(sgroups):
        sbins = []
        for kv, f in sorted(sgroups[sg], key=lambda x: -len(x[1])):
            for b in sbins:
                if sum(len(x[1]) for x in b) + len(f) <= CHUNK:
                    b.append((kv, f))
                    break
            else:
                sbins.append([(kv, f)])
        bins.extend(sbins)
    # merge small bins globally (saves chunks at the cost of 1-2 extra mms)
    bins.sort(key=lambda b: sum(len(x[1]) for x in b))
    merged = []
    while bins:
        b = bins.pop(0)
        size = sum(len(x[1]) for x in b)
        for i in range(len(bins) - 1, -1, -1):
            if sum(len(x[1]) for x in bins[i]) + size <= CHUNK:
                bins[i].extend(b)
                break
        else:
            merged.append(b)
    merged.sort(key=lambda b: min(x[0] for x in b))

    order = []
    chunks = []
    for b in merged:
        start = len(order)
        for kv, f in b:
            order.extend(f.tolist())
        chunks.append((start, len(order) - start))
    order = np.array(order)
    assert len(order) == F and len(np.unique(order)) == F

    items = [[[] for _ in range(NBLK)] for _ in range(DEG)]
    for ci, (start, fill) in enumerate(chunks):
        feats = order[start:start + fill]
        for d in range(DEG):
            for g in np.unique(blocks[d, feats]):
                g = int(g)
                rows = np.nonzero(blocks[d, feats] == g)[0]
                Z = np.zeros((CHUNK, 128), np.float32)
                Z[rows, idx[d, feats[rows]] - 128 * g] = sgn[d, feats[rows]]
                items[d][g].append((ci, Z))
    for d in range(DEG):
        for g in range(NBLK):
            if not items[d][g]:
                items[d][g].append((0, np.zeros((CHUNK, 128), np.float32)))

    zmats = []
    plan = [[[] for _ in range(NBLK)] for _ in range(DEG)]
    for d in range(DEG):
        for g in range(NBLK):
            for (ci, Z) in sorted(items[d][g], key=lambda x: x[0]):
                plan[d][g].append((ci, len(zmats)))
                zmats.append(Z)
    zm = np.stack(zmats)                                # [nmm, 128, 128]
    zm_t = np.ascontiguousarray(zm.transpose(1, 0, 2))  # [128, nmm, 128]
    return order, chunks, plan, zm_t


def build_dft_tables():
    n = np.arange(NCOMP)[:, None]
    k = np.arange(257)[None, :]
    ang = 2 * np.pi * n * k / NCOMP
    # stage-2 lhsT [512, 514]: cols 0..256 Re coeffs, cols 257..513 Im coeffs
    dft = np.concatenate([np.cos(ang), -np.sin(ang)], axis=1).astype(np.float32)
    dft_t = np.ascontiguousarray(
        dft.reshape(4, 128, 514).transpose(1, 0, 2))    # [128, 4, 514]

    kk = np.arange(257)[:, None]
    nn = np.arange(NCOMP)[None, :]
    ang2 = 2 * np.pi * kk * nn / NCOMP
    ck = np.full((257, 1), 2.0, np.float32); ck[0] = 1.0
    dk = np.full((257, 1), 2.0, np.float32); dk[0] = 0.0
    ire = (ck * np.cos(ang2) / NCOMP).astype(np.float32)   # [257, 512]
    iim = (-dk * np.sin(ang2) / NCOMP).astype(np.float32)
    ico = np.zeros((4, 128, NCOMP), np.float32)
    ico[0] = ire[0:128]
    ico[1] = ire[128:256]
    ico[2] = iim[0:128]
    ico[3] = iim[128:256]
    # Nyquist rides in the identically-zero Im(0) slot: its product lands in
    # prod[Im-block-A, row 0], and the matching irfft column is (1/N)(-1)^n.
    ico[2, 0] = np.cos(np.pi * np.arange(NCOMP)).astype(np.float32) / NCOMP
    ico_t = np.ascontiguousarray(ico.transpose(1, 0, 2))   # [128, 4, 512]
    return dft_t, ico_t


def build_program(plan, chunks, nmm, b_core=B_CORE):
    nbt = b_core // BT
    nch = len(chunks)
    ngrp = (nch + 7) // 8
    fills = [f for (_, f) in chunks]
    # (d, g) -> [lo, hi) slice into the z stack
    zoff = {}
    pos = 0
    for d in range(DEG):
        for g in range(NBLK):
            zoff[(d, g)] = (pos, pos + len(plan[d][g]))
            pos += len(plan[d][g])
    assert pos == nmm

    nc = bacc.Bacc("TRN2", target_bir_lowering=False, debug=False)
    xp = nc.dram_tensor("xp", [len(chunks) * 128, b_core], F32R,
                        kind="ExternalInput").ap()
    zm = nc.dram_tensor("zm", [128, nmm, 128], F32R, kind="ExternalInput").ap()
    dft = nc.dram_tensor("dft", [128, 4, 514], F32R, kind="ExternalInput").ap()
    ico = nc.dram_tensor("ico", [128, 4, 512], F32R, kind="ExternalInput").ap()
    ot = nc.dram_tensor("ot", [NCOMP, b_core], F32, kind="ExternalOutput").ap()

    with tile.TileContext(nc) as tc:
        with (
            tc.tile_pool(name="pz", bufs=1) as pz,
            tc.tile_pool(name="pc", bufs=1) as pc,
            tc.tile_pool(name="px", bufs=2) as px,
            tc.tile_pool(name="psk", bufs=1) as psk,
            tc.tile_pool(name="pprod", bufs=2) as pprod,
            tc.tile_pool(name="ptmp", bufs=1) as ptmp,
            tc.tile_pool(name="pout", bufs=4) as pout,
            tc.tile_pool(name="ps_sk", bufs=2, space="PSUM") as ps_sk,
            tc.tile_pool(name="ps_fr", bufs=3, space="PSUM") as ps_fr,
            tc.tile_pool(name="ps_out", bufs=3, space="PSUM") as ps_out,
        ):
            zts = {}
            consts = {}
            prods = {}

            def emit_stage4(tt):
                prod = prods.pop(tt)
                icot = consts["icot"]
                for m in range(4):
                    po = ps_out.tile([128, BT], F32, tag="po")
                    for q in range(4):
                        nc.tensor.matmul(
                            po[:], icot[:, q, 128 * m:128 * (m + 1)],
                            prod[:, q, :],
                            start=(q == 0), stop=(q == 3))
                    ob = pout.tile([128, BT], F32, tag="ob")
                    nc.vector.tensor_copy(ob[:], po[:])
                    nc.scalar.dma_start(
                        ot[128 * m:128 * (m + 1), BT * tt:BT * (tt + 1)], ob[:])

            def load_z(d, g):
                lo, hi = zoff[(d, g)]
                zt = pz.tile([128, hi - lo, 128], F32R, tag=f"z{d}{g}")
                nc.sync.dma_start(zt[:], zm[:, lo:hi, :])
                zts[(d, g)] = zt

            def load_xg(xgs, j, t):
                if j in xgs:
                    return
                w = min(8, nch - 8 * j)
                xt = px.tile([128, w, BT], F32R, tag=f"xg{j}")
                src = xp[1024 * j:1024 * j + 128 * w, BT * t:BT * (t + 1)]
                nc.sync.dma_start(xt[:], src.rearrange("(c p) n -> p c n", p=128))
                xgs[j] = xt

            for t in range(nbt):
                xgs = {}
                if t == 0:
                    # startup-latency-ordered loads: the (0, g) Z pieces and
                    # the X groups they touch arrive first so PE starts ~ASAP
                    for g in range(NBLK):
                        load_z(0, g)
                        for (ci, _) in plan[0][g]:
                            load_xg(xgs, ci // 8, t)
                    for j in range(ngrp):
                        load_xg(xgs, j, t)
                    dftt = pc.tile([128, 4, 514], F32R, tag="dftt")
                    nc.sync.dma_start(dftt[:], dft[:])
                    icot = pc.tile([128, 4, 512], F32R, tag="icot")
                    nc.sync.dma_start(icot[:], ico[:])
                    consts["dftt"] = dftt
                    consts["icot"] = icot
                    for d in (1, 2):
                        for g in range(NBLK):
                            load_z(d, g)
                else:
                    for j in range(ngrp):
                        load_xg(xgs, j, t)
                dftt = consts["dftt"]
                icot = consts["icot"]

                prod = pprod.tile([128, 4, BT], F32R, tag="prod")

                # ---- stage 1 for ALL degrees first: keeps the in-order PE
                # queue saturated with independent matmuls while ACT/DVE chew
                # on copies, and lets stage 2 find its SBUF inputs ready.
                sksd = []
                ssums = []
                for d in range(DEG):
                    sks = []
                    for g in range(NBLK):
                        pssk = ps_sk.tile([128, BT], F32, tag="psk")
                        items = plan[d][g]
                        zt = zts[(d, g)]
                        for i, (ci, zi) in enumerate(items):
                            fill = fills[ci]
                            nc.tensor.matmul(
                                pssk[:],
                                zt[0:fill, i, :],
                                xgs[ci // 8][0:fill, ci % 8, :],
                                start=(i == 0),
                                stop=(i == len(items) - 1),
                            )
                        sk = psk.tile([128, BT], F32R, tag=f"sk{d}{g}")
                        nc.scalar.copy(sk[:], pssk[:])
                        sks.append(sk)
                    sksd.append(sks)
                    # alternating-sign sum feeding the Nyquist bin:
                    # Re(256) = sum_p (-1)^p (sk0+sk1+sk2+sk3)[p]
                    s01 = ptmp.tile([128, BT], F32, tag="t1")
                    s23 = ptmp.tile([128, BT], F32, tag="t2")
                    ssum = ptmp.tile([128, BT], F32R, tag=f"ss{d}")
                    nc.vector.tensor_add(s01[:], sks[0][:].bitcast(F32),
                                         sks[1][:].bitcast(F32))
                    nc.vector.tensor_add(s23[:], sks[2][:].bitcast(F32),
                                         sks[3][:].bitcast(F32))
                    nc.vector.tensor_add(ssum[:], s01[:], s23[:])
                    ssums.append(ssum)

                # ---- stage 4 of the PREVIOUS btile: its product is long done
                if t > 0:
                    emit_stage4(t - 1)

                # ---- stage 2 + 3 per degree
                for d in range(DEG):
                    sks = sksd[d]
                    for pi, (mre, mim) in enumerate(((0, 257), (128, 385))):
                        frre = ps_fr.tile([128, BT], F32, tag="fr")
                        frim = ps_fr.tile([128, BT], F32, tag="fr")
                        for q in range(4):
                            nc.tensor.matmul(
                                frre[:], dftt[:, q, mre:mre + 128], sks[q][:],
                                start=(q == 0), stop=(q == 3))
                        for q in range(4):
                            nc.tensor.matmul(
                                frim[:], dftt[:, q, mim:mim + 128], sks[q][:],
                                start=(q == 0), stop=(q == 3))
                        if pi == 0:
                            # Nyquist row via the summed sketch (one matmul)
                            frt = ps_out.tile([1, BT], F32, tag="po")
                            nc.tensor.matmul(
                                frt[:], dftt[:, 0, 256:257], ssums[d][:],
                                start=True, stop=True)
                            pret = prod[0:1, 2, :]
                            tT = None
                            if d > 0:
                                tT = ptmp.tile([1, BT], F32R, tag="tT")
                                nc.vector.tensor_mul(
                                    tT[:], pret.bitcast(F32), frt[:])
                        pre = prod[:, pi, :]
                        pim = prod[:, 2 + pi, :]
                        if d == 0:
                            nc.vector.tensor_copy(pre, frre[:])
                            nc.vector.tensor_copy(pim, frim[:])
                        else:
                            pre32 = pre.bitcast(F32)
                            pim32 = pim.bitcast(F32)
                            t1 = ptmp.tile([128, BT], F32, tag="t1")
                            t2 = ptmp.tile([128, BT], F32, tag="t2")
                            t3 = ptmp.tile([128, BT], F32, tag="t3")
                            t4 = ptmp.tile([128, BT], F32, tag="t4")
                            nc.vector.tensor_mul(t1[:], pre32, frre[:])
                            nc.vector.tensor_mul(t2[:], pim32, frim[:])
                            nc.vector.tensor_mul(t3[:], pre32, frim[:])
                            nc.vector.tensor_mul(t4[:], pim32, frre[:])
                            nc.vector.tensor_sub(pre, t1[:], t2[:])
                            nc.vector.tensor_add(pim, t3[:], t4[:])
                    if tT is not None:
                        nc.vector.tensor_copy(pret, tT[:])
                    else:
                        nc.vector.tensor_copy(pret, frt[:])
                prods[t] = prod
            emit_stage4(nbt - 1)

    nc.compile()
    return nc


def prepare_inputs(X, index_hash, bit_hash):
    order, chunks, plan, zm_t = build_plan(index_hash, bit_hash)
    dft_t, ico_t = build_dft_tables()
    nmm = zm_t.shape[1]
    # padded layout: chunk c occupies rows [128c, 128c+fill), rest zero
    Xt = round_fp32r(np.asarray(X, np.float32).T[order])
    Xp = np.zeros((len(chunks) * 128, Xt.shape[1]), np.float32)
    for c, (start, fill) in enumerate(chunks):
        Xp[128 * c:128 * c + fill] = Xt[start:start + fill]
    shared = {
        "zm": zm_t,                      # +-1/0: exact in fp32r already
        "dft": round_fp32r(dft_t),
        "ico": round_fp32r(ico_t),
    }
    return plan, chunks, nmm, Xp, shared


def kernel(X, index_hash, bit_hash, _trace=False):
    plan, chunks, nmm, Xp, shared = prepare_inputs(X, index_hash, bit_hash)
    nc = build_program(plan, chunks, nmm)
    in_maps = [
        {"xp": np.ascontiguousarray(Xp[:, c * B_CORE:(c + 1) * B_CORE]), **shared}
        for c in range(NCORES)
    ]
    res = bass_utils.run_bass_kernel_spmd(
        nc, in_maps, core_ids=list(range(NCORES)), trace=_trace)
    out = np.empty((B, NCOMP), np.float32)
    for c in range(NCORES):
        out[c * B_CORE:(c + 1) * B_CORE] = res.results[c]["ot"].T
    return (out, res) if _trace else out



# revision 13
# speedup vs baseline: 1.2859x; 1.2859x over previous
"""PolyCntSketch (TensorSketch, degree 3) Trainium2 kernel.

Math: for each degree d, CountSketch_d = X @ S_d (S_d one-hot signed), then
out = irfft(prod_d rfft(CountSketch_d)).

Device strategy (pure data parallelism over batch, 8 cores, B_core = 1024):
  - Host feeds X transposed ([F, B_core]) in fp16, features packed into
    128-row chunks where each chunk holds whole (block_d0, block_d1,
    block_d2)-classes (block = idx_d // 128), so each chunk touches few
    128-bucket blocks per degree -> few segment matmuls.
  - Stage 1 (weight-stationary, full batch): per (degree, block) the plan's
    Z one-hot matrices accumulate sketch PSUM [128, 512] x 2 half-batches.
    PSUM drains to fp16 SBUF via scalar/gpsimd/vector round-robin.
  - Stage 2: rfft as DFT matmul (512 -> 257 complex), fp16 weights. The
    Nyquist bin Re(256) rides in the identically-zero Im(0) column of the
    DFT matrix (patched to the alternating +-1 column), so it needs no
    extra matmuls; the DC and Nyquist rows of the complex product are
    fixed up with [1, 512] ops at the end of each degree chain.
  - Stage 3: complex product across the 3 degrees, all fp16 (2x DVE rate).
    DFT is scaled by 1/32 (exact) so the fp16 product cannot overflow;
    the irfft table is scaled by 2^15 to compensate.
  - Stage 4: irfft as matmul -> out^T [512, B_core] f32.
"""
import sys

for _p in ("/opt/trn_rl_repo",):
    if _p not in sys.path:
        sys.path.append(_p)

import numpy as np

from concourse import bacc, mybir, tile
from concourse import bass_utils

F16 = mybir.dt.float16
F32 = mybir.dt.float32

B, F, NCOMP, DEG = 8192, 4096, 512, 3
NCORES = 8
B_CORE = B // NCORES
BT = 512                     # batch columns per matmul (PSUM bank = 512 f32)
NBT = B_CORE // BT           # 2 half-batches
CHUNK = 128
NBLK = NCOMP // 128          # 4 bucket blocks
DFT_SCALE = 1.0 / 32.0       # exact power of two; keeps fp16 products small


def build_plan(index_hash, bit_hash):
    """FFD-pack whole (g0,g1,g2)-classes into 128-row chunks.

    Returns:
      order [F]: feature order for the transposed X upload
      chunks: list of (start, fill) row ranges into the ordered X
      plan[d][g]: list of (chunk_idx, zslot) in emission order ((d,g)-major)
      zm_t [128, nmm, 128]: stacked Z matrices, partition-major
    """
    idx = np.asarray(index_hash)
    sgn = (np.asarray(bit_hash) * 2 - 1).astype(np.float32)
    blocks = idx >> 7
    key = blocks[0] * 16 + blocks[1] * 4 + blocks[2]
    order_all = np.argsort(key, kind="stable")
    kvals = key[order_all]

    from collections import defaultdict
    sgroups = defaultdict(list)
    for kv in np.unique(kvals):
        f = order_all[kvals == kv]
        kv = int(kv)
        while len(f) > CHUNK:
            sgroups[kv >> 2].append((kv, f[:CHUNK]))
            f = f[CHUNK:]
        if len(f):
            sgroups[kv >> 2].append((kv, f))

    bins = []
    for sg in sorted(sgroups):
        sbins = []
        for kv, f in sorted(sgroups[sg], key=lambda x: -len(x[1])):
            for b in sbins:
                if sum(len(x[1]) for x in b) + len(f) <= CHUNK:
                    b.append((kv, f))
                    break
            else:
                sbins.append([(kv, f)])
        bins.extend(sbins)
    # merge small bins globally (saves chunks at the cost of 1-2 extra mms)
    bins.sort(key=lambda b: sum(len(x[1]) for x in b))
    merged = []
    while bins:
        b = bins.pop(0)
        size = sum(len(x[1]) for x in b)
        for i in range(len(bins) - 1, -1, -1):
            if sum(len(x[1]) for x in bins[i]) + size <= CHUNK:
                bins[i].extend(b)
                break
        else:
            merged.append(b)
    merged.sort(key=lambda b: min(x[0] for x in b))

    order = []
    chunks = []
    for b in merged:
        start = len(order)
        for kv, f in b:
            order.extend(f.tolist())
        chunks.append((start, len(order) - start))
    order = np.array(order)
    assert len(order) == F and len(np.unique(order)) == F

    items = [[[] for _ in range(NBLK)] for _ in range(DEG)]
    for ci, (start, fill) in enumerate(chunks):
        feats = order[start:start + fill]
        for d in range(DEG):
            for g in np.unique(blocks[d, feats]):
                g = int(g)
                rows = np.nonzero(blocks[d, feats] == g)[0]
                Z = np.zeros((CHUNK, 128), np.float32)
                Z[rows, idx[d, feats[rows]] - 128 * g] = sgn[d, feats[rows]]
                items[d][g].append((ci, Z))
    for d in range(DEG):
        for g in range(NBLK):
            if not items[d][g]:
                items[d][g].append((0, np.zeros((CHUNK, 128), np.float32)))

    zmats = []
    plan = [[[] for _ in range(NBLK)] for _ in range(DEG)]
    for d in range(DEG):
        for g in range(NBLK):
            for (ci, Z) in sorted(items[d][g], key=lambda x: x[0]):
                plan[d][g].append((ci, len(zmats)))
                zmats.append(Z)
    zm = np.stack(zmats)                                # [nmm, 128, 128]
    zm_t = np.ascontiguousarray(zm.transpose(1, 0, 2))  # [128, nmm, 128]
    return order, chunks, plan, zm_t


def build_dft_tables():
    n = np.arange(NCOMP)[:, None]
    k = np.arange(257)[None, :]
    ang = 2 * np.pi * n * k / NCOMP
    # stage-2 lhsT [512, 514]: cols 0..256 Re coeffs, cols 257..513 Im coeffs.
    # Col 257 is Im(0) == 0: replace it with the Nyquist column (-1)^n so
    # Re(256) rides in the Im(0) slot for free.
    dft = np.concatenate([np.cos(ang), -np.sin(ang)], axis=1)
    dft[:, 257] = np.cos(np.pi * np.arange(NCOMP))
    dft = (dft * DFT_SCALE).astype(np.float32)
    dft_t = np.ascontiguousarray(
        dft.reshape(4, 128, 514).transpose(1, 0, 2))    # [128, 4, 514]

    kk = np.arange(257)[:, None]
    nn = np.arange(NCOMP)[None, :]
    ang2 = 2 * np.pi * kk * nn / NCOMP
    ck = np.full((257, 1), 2.0, np.float32); ck[0] = 1.0
    dk = np.full((257, 1), 2.0, np.float32); dk[0] = 0.0
    iscale = 1.0 / DFT_SCALE ** 3
    ire = (ck * np.cos(ang2) / NCOMP) * iscale           # [257, 512]
    iim = (-dk * np.sin(ang2) / NCOMP) * iscale
    ico = np.zeros((4, 128, NCOMP), np.float32)
    ico[0] = ire[0:128]
    ico[1] = ire[128:256]
    ico[2] = iim[0:128]
    ico[3] = iim[128:256]
    # Nyquist rides in the Im(0) slot; its irfft column is (1/N)(-1)^n.
    ico[2, 0] = np.cos(np.pi * np.arange(NCOMP)) / NCOMP * iscale
    ico_t = np.ascontiguousarray(ico.transpose(1, 0, 2))   # [128, 4, 512]
    return dft_t, ico_t


def build_program(plan, chunks, nmm, b_core=B_CORE):
    nch = len(chunks)
    fills = [f for (_, f) in chunks]
    # (d, g) -> [lo, hi) slice into the z stack
    zoff = {}
    pos = 0
    for d in range(DEG):
        for g in range(NBLK):
            zoff[(d, g)] = (pos, pos + len(plan[d][g]))
            pos += len(plan[d][g])
    assert pos == nmm

    nc = bacc.Bacc("TRN2", target_bir_lowering=False, debug=False)
    xp = nc.dram_tensor("xp", [nch * 128, b_core], F16,
                        kind="ExternalInput").ap()
    zm = nc.dram_tensor("zm", [128, nmm, 128], F16, kind="ExternalInput").ap()
    dft = nc.dram_tensor("dft", [128, 4, 514], F16, kind="ExternalInput").ap()
    ico = nc.dram_tensor("ico", [128, 4, 512], F16, kind="ExternalInput").ap()
    ot = nc.dram_tensor("ot", [NCOMP, b_core], F32, kind="ExternalOutput").ap()

    with tile.TileContext(nc) as tc:
        with (
            tc.tile_pool(name="pz", bufs=1) as pz,
            tc.tile_pool(name="pc", bufs=1) as pc,
            tc.tile_pool(name="px", bufs=1) as px,
            tc.tile_pool(name="psk", bufs=1) as psk,
            tc.tile_pool(name="pprod", bufs=1) as pprod,
            tc.tile_pool(name="pfr", bufs=2) as pfr,
            tc.tile_pool(name="ptmp", bufs=2) as ptmp,
            tc.tile_pool(name="prow", bufs=2) as prow,
            tc.tile_pool(name="pout", bufs=2) as pout,
            tc.tile_pool(name="ps_sk", bufs=2, space="PSUM") as ps_sk,
            tc.tile_pool(name="ps_fr", bufs=2, space="PSUM") as ps_fr,
        ):
            xts = {}
            zts = {}

            def load_z(d, g):
                lo, hi = zoff[(d, g)]
                zt = pz.tile([128, hi - lo, 128], F16, tag=f"z{d}{g}")
                nc.sync.dma_start(zt[:], zm[:, lo:hi, :])
                zts[(d, g)] = zt

            def load_x(ci):
                if ci in xts:
                    return
                xt = px.tile([128, b_core], F16, tag=f"x{ci}")
                nc.sync.dma_start(xt[:], xp[128 * ci:128 * (ci + 1), :])
                xts[ci] = xt

            # ---- DMA: z for degree 0, X chunks in first-use order, then the
            # later-needed z/const tables trickling in just ahead of use.
            # Fine-grained per-chunk tiles let stage-1 start as soon as the
            # first chunks land; X dominates bytes so it goes first.
            for g in range(NBLK):
                load_z(0, g)
            for g in range(NBLK):
                for (ci, _) in plan[0][g]:
                    load_x(ci)
            for ci in range(nch):
                load_x(ci)
            load_z(1, 0)
            load_z(1, 1)
            dftt = pc.tile([128, 4, 514], F16, tag="dftt")
            nc.sync.dma_start(dftt[:], dft[:])
            load_z(1, 2)
            load_z(1, 3)
            icot = pc.tile([128, 4, 512], F16, tag="icot")
            nc.sync.dma_start(icot[:], ico[:])
            for g in range(NBLK):
                load_z(2, g)

            drain_engines = [nc.scalar, nc.gpsimd, nc.vector]

            # ---- stage 1: weight-stationary count sketch over full batch.
            sks = {}

            def stage1(d, g, di):
                items = plan[d][g]
                zt = zts[(d, g)]
                ps = [ps_sk.tile([128, BT], F32, tag=f"skh{h}", name=f"ps{h}")
                      for h in range(NBT)]
                for i, (ci, zi) in enumerate(items):
                    fill = fills[ci]
                    zslice = zt[0:fill, i, :]
                    st = (i == 0)
                    sp = (i == len(items) - 1)
                    for h in range(NBT):
                        nc.tensor.matmul(
                            ps[h][:], zslice,
                            xts[ci][0:fill, BT * h:BT * (h + 1)],
                            start=st, stop=sp)
                sk = psk.tile([128, b_core], F16, tag=f"sk{d}{g}")
                # PSUM readers are limited to ACT/DVE; alternate them.
                nc.scalar.copy(sk[:, 0:BT], ps[0][:])
                nc.vector.tensor_copy(sk[:, BT:2 * BT], ps[1][:])
                sks[(d, g)] = sk

            # ---- stage 2+3 for one (degree, half, pi): 8 matmuls -> fp16
            # cast -> complex product chain into prod[h].
            prods = {}
            rowfix = {}

            def get_prod(h):
                if h not in prods:
                    prods[h] = pprod.tile([128, 4, BT], F16, tag=f"prod{h}",
                                          name=f"prod{h}")
                return prods[h]

            def stage23(d, h, pi):
                prod = get_prod(h)
                mre = 128 * pi
                mim = 257 + 128 * pi
                frre = ps_fr.tile([128, BT], F32, tag="re")
                frim = ps_fr.tile([128, BT], F32, tag="im")
                for q in range(4):
                    nc.tensor.matmul(
                        frre[:], dftt[:, q, mre:mre + 128],
                        sks[(d, q)][:, BT * h:BT * (h + 1)],
                        start=(q == 0), stop=(q == 3))
                for q in range(4):
                    nc.tensor.matmul(
                        frim[:], dftt[:, q, mim:mim + 128],
                        sks[(d, q)][:, BT * h:BT * (h + 1)],
                        start=(q == 0), stop=(q == 3))
                pre = prod[:, pi, :]
                pim = prod[:, 2 + pi, :]
                if d == 0:
                    # cast PSUM f32 -> fp16 directly into prod (ACT engine;
                    # GPSIMD cannot read PSUM)
                    nc.scalar.copy(pre, frre[:])
                    nc.scalar.copy(pim, frim[:])
                    if pi == 0:
                        dc = prow.tile([1, BT], F16, tag=f"dc{h}")
                        ny = prow.tile([1, BT], F16, tag=f"ny{h}")
                        nc.scalar.copy(dc[:], frre[0:1, :])
                        nc.scalar.copy(ny[:], frim[0:1, :])
                        rowfix[h] = (dc, ny)
                else:
                    fre = pfr.tile([128, BT], F16, tag="fre")
                    fim = pfr.tile([128, BT], F16, tag="fim")
                    nc.scalar.copy(fre[:], frre[:])
                    nc.scalar.copy(fim[:], frim[:])
                    if pi == 0:
                        dc0, ny0 = rowfix[h]
                        dc = prow.tile([1, BT], F16, tag=f"dc{h}")
                        ny = prow.tile([1, BT], F16, tag=f"ny{h}")
                        nc.vector.tensor_mul(dc[:], dc0[:], fre[0:1, :])
                        nc.vector.tensor_mul(ny[:], ny0[:], fim[0:1, :])
                        rowfix[h] = (dc, ny)
                    t1 = ptmp.tile([128, BT], F16, tag="t1")
                    t2 = ptmp.tile([128, BT], F16, tag="t2")
                    t3 = ptmp.tile([128, BT], F16, tag="t3")
                    t4 = ptmp.tile([128, BT], F16, tag="t4")
                    nc.vector.tensor_mul(t1[:], pre, fre[:])
                    nc.gpsimd.tensor_mul(t2[:], pim, fim[:])
                    nc.vector.tensor_mul(t3[:], pre, fim[:])
                    nc.gpsimd.tensor_mul(t4[:], pim, fre[:])
                    nc.vector.tensor_sub(pre, t1[:], t2[:])
                    nc.gpsimd.tensor_add(pim, t3[:], t4[:])
                    if d == DEG - 1 and pi == 0:
                        dc, ny = rowfix[h]
                        nc.vector.tensor_copy(prod[0:1, 0, :], dc[:])
                        nc.vector.tensor_copy(prod[0:1, 2, :], ny[:])

            # ---- stage 4: irfft as matmul, drain via SBUF f32, DMA out.
            def stage4(h):
                prod = prods[h]
                for m in range(4):
                    po = ps_sk.tile([128, BT], F32, tag=f"skh{m % 2}")
                    for q in range(4):
                        nc.tensor.matmul(
                            po[:], icot[:, q, 128 * m:128 * (m + 1)],
                            prod[:, q, :],
                            start=(q == 0), stop=(q == 3))
                    ob = pout.tile([128, BT], F32, tag=f"ob{m % 2}")
                    if m % 2 == 0:
                        nc.scalar.copy(ob[:], po[:])
                    else:
                        nc.vector.tensor_copy(ob[:], po[:])
                    nc.scalar.dma_start(
                        ot[128 * m:128 * (m + 1), BT * h:BT * (h + 1)], ob[:])

            # ---- emission schedule: keep the PE queue saturated; let the
            # pointwise engines run stage-3 chains behind stage-2 matmuls.
            di = 0
            for g in range(NBLK):
                stage1(0, g, di); di += 1
            for g in range(NBLK):
                stage1(1, g, di); di += 1
            for pi in range(2):
                stage23(0, 0, pi)
            for g in range(NBLK):
                stage1(2, g, di); di += 1
            for pi in range(2):
                stage23(0, 1, pi)
            for pi in range(2):
                stage23(1, 0, pi)
            for pi in range(2):
                stage23(2, 0, pi)
            for pi in range(2):
                stage23(1, 1, pi)
            stage4(0)
            for pi in range(2):
                stage23(2, 1, pi)
            stage4(1)

    nc.compile()
    return nc


def round_fp16(x):
    return np.asarray(x, np.float32).astype(np.float16)


def prepare_inputs(X, index_hash, bit_hash):
    order, chunks, plan, zm_t = build_plan(index_hash, bit_hash)
    dft_t, ico_t = build_dft_tables()
    nmm = zm_t.shape[1]
    # padded layout: chunk c occupies rows [128c, 128c+fill), rest zero
    Xt = np.asarray(X, np.float32).T[order]
    Xp = np.zeros((len(chunks) * 128, Xt.shape[1]), np.float16)
    for c, (start, fill) in enumerate(chunks):
        Xp[128 * c:128 * c + fill] = Xt[start:start + fill]
    shared = {
        "zm": round_fp16(zm_t),          # +-1/0: exact in fp16
        "dft": round_fp16(dft_t),
        "ico": round_fp16(ico_t),
    }
    return plan, chunks, nmm, Xp, shared


def kernel(X, index_hash, bit_hash, _trace=False):
    plan, chunks, nmm, Xp, shared = prepare_inputs(X, index_hash, bit_hash)
    nc = build_program(plan, chunks, nmm)
    in_maps = [
        {"xp": np.ascontiguousarray(Xp[:, c * B_CORE:(c + 1) * B_CORE]), **shared}
        for c in range(NCORES)
    ]
    res = bass_utils.run_bass_kernel_spmd(
        nc, in_maps, core_ids=list(range(NCORES)), trace=_trace)
    out = np.empty((B, NCOMP), np.float32)
    for c in range(NCORES):
        out[c * B_CORE:(c + 1) * B_CORE] = res.results[c]["ot"].T
    return (out, res) if _trace else out


# revision 17
# speedup vs baseline: 1.5749x; 1.2248x over previous
"""PolyCntSketch (TensorSketch, degree 3) Trainium2 kernel.

Math: for each degree d, CountSketch_d = X @ S_d (S_d one-hot signed), then
out = irfft(prod_d rfft(CountSketch_d)).

Device strategy (pure data parallelism over batch, 8 cores, B_core = 1024):
  - Host feeds X transposed ([F, B_core]) in fp16, features packed into
    128-row chunks where each chunk holds whole (block_d0, block_d1,
    block_d2)-classes (block = idx_d // 128), so each chunk touches few
    128-bucket blocks per degree -> few segment matmuls.
  - Stage 1 (weight-stationary, full batch): per (degree, block) the plan's
    Z one-hot matrices accumulate sketch PSUM [128, 512] x 2 half-batches.
    PSUM drains to fp16 SBUF via scalar/gpsimd/vector round-robin.
  - Stage 2: rfft as DFT matmul (512 -> 257 complex), fp16 weights. The
    Nyquist bin Re(256) rides in the identically-zero Im(0) column of the
    DFT matrix (patched to the alternating +-1 column), so it needs no
    extra matmuls; the DC and Nyquist rows of the complex product are
    fixed up with [1, 512] ops at the end of each degree chain.
  - Stage 3: complex product across the 3 degrees, all fp16 (2x DVE rate).
    DFT is scaled by 1/32 (exact) so the fp16 product cannot overflow;
    the irfft table is scaled by 2^15 to compensate.
  - Stage 4: irfft as matmul -> out^T [512, B_core] f32.
"""
import sys

for _p in ("/opt/trn_rl_repo",):
    if _p not in sys.path:
        sys.path.append(_p)

import numpy as np

from concourse import bacc, mybir, tile
from concourse import bass_utils

F16 = mybir.dt.float16
F32 = mybir.dt.float32

B, F, NCOMP, DEG = 8192, 4096, 512, 3
NCORES = 8
B_CORE = B // NCORES
BT = 512                     # batch columns per matmul (PSUM bank = 512 f32)
NBT = B_CORE // BT           # 2 half-batches
CHUNK = 128
NBLK = NCOMP // 128          # 4 bucket blocks
DFT_SCALE = 1.0 / 32.0       # exact power of two; keeps fp16 products small


def _pack_classes(kvs_counts, seed_count=8, iters=60000):
    """Pack (g0,g1,g2)-classes into <=128-row bins, minimizing the total
    matmul count sum_bins sum_d #distinct-blocks. Greedy first-fit by
    marginal cost + hill climbing with move/swap steps."""
    import random

    def cost_of(binkvs):
        if not binkvs:
            return 0
        return sum(len(set((kv >> sh) & 3 for kv in binkvs))
                   for sh in (4, 2, 0))

    items = []
    for kv, s in kvs_counts:
        while s > CHUNK:
            items.append((kv, CHUNK)); s -= CHUNK
        if s:
            items.append((kv, s))

    def greedy(order_classes):
        bins, sizes = [], []
        for kv, s in order_classes:
            best, bestdelta = None, None
            for i, b in enumerate(bins):
                if sizes[i] + s <= CHUNK:
                    delta = (cost_of([k for k, _ in b] + [kv])
                             - cost_of([k for k, _ in b]))
                    if bestdelta is None or delta < bestdelta:
                        best, bestdelta = i, delta
            if best is not None and bestdelta <= 1:
                bins[best].append((kv, s)); sizes[best] += s
            else:
                bins.append([(kv, s)]); sizes.append(s)
        return bins, sizes

    def hill(bins, sizes, seed):
        rng = random.Random(seed)
        bins = [list(b) for b in bins]; sizes = list(sizes)

        def bc(i):
            return cost_of([k for k, _ in bins[i]])

        for _ in range(iters):
            r = rng.random()
            i = rng.randrange(len(bins)); j = rng.randrange(len(bins))
            if i == j or not bins[i]:
                continue
            if r < 0.6:
                ii = rng.randrange(len(bins[i])); kv, s = bins[i][ii]
                if sizes[j] + s > CHUNK:
                    continue
                cb = bc(i) + bc(j)
                bi2 = [x for xi, x in enumerate(bins[i]) if xi != ii]
                ca = (cost_of([k for k, _ in bi2])
                      + cost_of([k for k, _ in bins[j]] + [kv]))
                if ca <= cb:
                    bins[i].pop(ii); bins[j].append((kv, s))
                    sizes[i] -= s; sizes[j] += s
            else:
                if not bins[j]:
                    continue
                ii = rng.randrange(len(bins[i])); jj = rng.randrange(len(bins[j]))
                kv1, s1 = bins[i][ii]; kv2, s2 = bins[j][jj]
                if sizes[i] - s1 + s2 > CHUNK or sizes[j] - s2 + s1 > CHUNK:
                    continue
                cb = bc(i) + bc(j)
                bi2 = [x for xi, x in enumerate(bins[i]) if xi != ii] + [(kv2, s2)]
                bj2 = [x for xj, x in enumerate(bins[j]) if xj != jj] + [(kv1, s1)]
                ca = (cost_of([k for k, _ in bi2])
                      + cost_of([k for k, _ in bj2]))
                if ca <= cb:
                    bins[i][ii] = (kv2, s2); bins[j][jj] = (kv1, s1)
                    sizes[i] += s2 - s1; sizes[j] += s1 - s2
        bins = [b for b in bins if b]
        return bins, sum(cost_of([k for k, _ in b]) for b in bins)

    best = None
    for seed in range(seed_count):
        o = items[:]
        random.Random(seed).shuffle(o)
        if seed % 2 == 0:
            o.sort(key=lambda x: (x[0] >> 2,))
        bins, sizes = greedy(o)
        bins, c = hill(bins, sizes, seed)
        if best is None or c < best[0]:
            best = (c, [list(b) for b in bins])
    return best[1]


def build_plan(index_hash, bit_hash):
    """Pack whole (g0,g1,g2)-classes into 128-row chunks minimizing the
    count of per-(chunk, degree, block) matmuls.

    Returns:
      order [F]: feature order for the transposed X upload
      chunks: list of (start, fill) row ranges into the ordered X
      plan[d][g]: list of (chunk_idx, zslot) in emission order ((d,g)-major)
      zm_t [128, nmm, 128]: stacked Z matrices, partition-major
    """
    idx = np.asarray(index_hash)
    sgn = (np.asarray(bit_hash) * 2 - 1).astype(np.float32)
    blocks = idx >> 7
    key = blocks[0] * 16 + blocks[1] * 4 + blocks[2]

    kvs, counts = np.unique(key, return_counts=True)
    bins = _pack_classes(sorted(zip(kvs.tolist(), counts.tolist())))

    # features per class, consumed front-to-back as bins reference (possibly
    # split) classes
    feat_of = {int(kv): np.nonzero(key == kv)[0].tolist() for kv in kvs}
    order = []
    chunks = []
    for b in bins:
        start = len(order)
        for kv, s in b:
            take = feat_of[kv][:s]
            feat_of[kv] = feat_of[kv][s:]
            order.extend(take)
        chunks.append((start, len(order) - start))
    order = np.array(order)
    assert len(order) == F and len(np.unique(order)) == F

    items = [[[] for _ in range(NBLK)] for _ in range(DEG)]
    for ci, (start, fill) in enumerate(chunks):
        feats = order[start:start + fill]
        for d in range(DEG):
            for g in np.unique(blocks[d, feats]):
                g = int(g)
                rows = np.nonzero(blocks[d, feats] == g)[0]
                Z = np.zeros((CHUNK, 128), np.float32)
                Z[rows, idx[d, feats[rows]] - 128 * g] = sgn[d, feats[rows]]
                items[d][g].append((ci, Z))
    for d in range(DEG):
        for g in range(NBLK):
            if not items[d][g]:
                items[d][g].append((0, np.zeros((CHUNK, 128), np.float32)))

    zmats = []
    plan = [[[] for _ in range(NBLK)] for _ in range(DEG)]
    for d in range(DEG):
        for g in range(NBLK):
            for (ci, Z) in sorted(items[d][g], key=lambda x: x[0]):
                plan[d][g].append((ci, len(zmats)))
                zmats.append(Z)
    zm = np.stack(zmats)                                # [nmm, 128, 128]
    zm_t = np.ascontiguousarray(zm.transpose(1, 0, 2))  # [128, nmm, 128]
    return order, chunks, plan, zm_t


def build_dft_tables():
    n = np.arange(NCOMP)[:, None]
    k = np.arange(257)[None, :]
    ang = 2 * np.pi * n * k / NCOMP
    # stage-2 lhsT [512, 514]: cols 0..256 Re coeffs, cols 257..513 Im coeffs.
    # Col 257 is Im(0) == 0: replace it with the Nyquist column (-1)^n so
    # Re(256) rides in the Im(0) slot for free.
    dft = np.concatenate([np.cos(ang), -np.sin(ang)], axis=1)
    dft[:, 257] = np.cos(np.pi * np.arange(NCOMP))
    dft = (dft * DFT_SCALE).astype(np.float32)
    dft_t = np.ascontiguousarray(
        dft.reshape(4, 128, 514).transpose(1, 0, 2))    # [128, 4, 514]

    kk = np.arange(257)[:, None]
    nn = np.arange(NCOMP)[None, :]
    ang2 = 2 * np.pi * kk * nn / NCOMP
    ck = np.full((257, 1), 2.0, np.float32); ck[0] = 1.0
    dk = np.full((257, 1), 2.0, np.float32); dk[0] = 0.0
    iscale = 1.0 / DFT_SCALE ** 3
    ire = (ck * np.cos(ang2) / NCOMP) * iscale           # [257, 512]
    iim = (-dk * np.sin(ang2) / NCOMP) * iscale
    ico = np.zeros((4, 128, NCOMP), np.float32)
    ico[0] = ire[0:128]
    ico[1] = ire[128:256]
    ico[2] = iim[0:128]
    ico[3] = iim[128:256]
    # Nyquist rides in the Im(0) slot; its irfft column is (1/N)(-1)^n.
    ico[2, 0] = np.cos(np.pi * np.arange(NCOMP)) / NCOMP * iscale
    ico_t = np.ascontiguousarray(ico.transpose(1, 0, 2))   # [128, 4, 512]
    return dft_t, ico_t


def build_program(plan, chunks, nmm, b_core=B_CORE):
    nch = len(chunks)
    fills = [f for (_, f) in chunks]
    # (d, g) -> [lo, hi) slice into the z stack
    zoff = {}
    pos = 0
    for d in range(DEG):
        for g in range(NBLK):
            zoff[(d, g)] = (pos, pos + len(plan[d][g]))
            pos += len(plan[d][g])
    assert pos == nmm

    nc = bacc.Bacc("TRN2", target_bir_lowering=False, debug=False)
    xp = nc.dram_tensor("xp", [nch * 128, b_core], F16,
                        kind="ExternalInput").ap()
    zm = nc.dram_tensor("zm", [128, nmm, 128], F16, kind="ExternalInput").ap()
    dft = nc.dram_tensor("dft", [128, 4, 514], F16, kind="ExternalInput").ap()
    ico = nc.dram_tensor("ico", [128, 4, 512], F16, kind="ExternalInput").ap()
    ot = nc.dram_tensor("ot", [NCOMP, b_core], F32, kind="ExternalOutput").ap()

    with tile.TileContext(nc) as tc:
        with (
            tc.tile_pool(name="pz", bufs=1) as pz,
            tc.tile_pool(name="pc", bufs=1) as pc,
            tc.tile_pool(name="px", bufs=1) as px,
            tc.tile_pool(name="psk", bufs=1) as psk,
            tc.tile_pool(name="pprod", bufs=1) as pprod,
            tc.tile_pool(name="pfr", bufs=2) as pfr,
            tc.tile_pool(name="ptmp", bufs=2) as ptmp,
            tc.tile_pool(name="prow", bufs=2) as prow,
            tc.tile_pool(name="pout", bufs=2) as pout,
            tc.tile_pool(name="ps_sk", bufs=2, space="PSUM") as ps_sk,
            tc.tile_pool(name="ps_fr", bufs=2, space="PSUM") as ps_fr,
        ):
            xts = {}
            zts = {}

            def load_z(d, g):
                lo, hi = zoff[(d, g)]
                zt = pz.tile([128, hi - lo, 128], F16, tag=f"z{d}{g}")
                nc.sync.dma_start(zt[:], zm[:, lo:hi, :])
                zts[(d, g)] = zt

            def load_x(ci):
                if ci in xts:
                    return
                xt = px.tile([128, b_core], F16, tag=f"x{ci}")
                nc.sync.dma_start(xt[:], xp[128 * ci:128 * (ci + 1), :])
                xts[ci] = xt

            # ---- DMA: z(0,g) interleaved with its own chunks so the first
            # stage-1 matmul isn't stuck behind later groups' z tables; then
            # the later-needed z/const tables trickle in just ahead of use.
            for g in range(NBLK):
                load_z(0, g)
                for (ci, _) in plan[0][g]:
                    load_x(ci)
            for ci in range(nch):
                load_x(ci)
            load_z(1, 0)
            load_z(1, 1)
            dftt = pc.tile([128, 4, 514], F16, tag="dftt")
            nc.sync.dma_start(dftt[:], dft[:])
            load_z(1, 2)
            load_z(1, 3)
            icot = pc.tile([128, 4, 512], F16, tag="icot")
            nc.sync.dma_start(icot[:], ico[:])
            for g in range(NBLK):
                load_z(2, g)

            drain_engines = [nc.scalar, nc.gpsimd, nc.vector]

            # ---- stage 1: weight-stationary count sketch over full batch.
            sks = {}

            def stage1(d, g, di):
                items = plan[d][g]
                zt = zts[(d, g)]
                ps = [ps_sk.tile([128, BT], F32, tag=f"skh{h}", name=f"ps{h}")
                      for h in range(NBT)]
                for i, (ci, zi) in enumerate(items):
                    fill = fills[ci]
                    zslice = zt[0:fill, i, :]
                    st = (i == 0)
                    sp = (i == len(items) - 1)
                    for h in range(NBT):
                        nc.tensor.matmul(
                            ps[h][:], zslice,
                            xts[ci][0:fill, BT * h:BT * (h + 1)],
                            start=st, stop=sp)
                sk = psk.tile([128, b_core], F16, tag=f"sk{d}{g}")
                # PSUM readers are limited to ACT/DVE; alternate them.
                nc.scalar.copy(sk[:, 0:BT], ps[0][:])
                nc.vector.tensor_copy(sk[:, BT:2 * BT], ps[1][:])
                sks[(d, g)] = sk

            # ---- stage 2+3 for one (degree, half, pi): 8 matmuls -> fp16
            # cast -> complex product chain into prod[h].
            prods = {}
            rowfix = {}

            def get_prod(h):
                if h not in prods:
                    prods[h] = pprod.tile([128, 4, BT], F16, tag=f"prod{h}",
                                          name=f"prod{h}")
                return prods[h]

            def stage23(d, h, pi):
                prod = get_prod(h)
                mre = 128 * pi
                mim = 257 + 128 * pi
                frre = ps_fr.tile([128, BT], F32, tag="re")
                frim = ps_fr.tile([128, BT], F32, tag="im")
                for q in range(4):
                    nc.tensor.matmul(
                        frre[:], dftt[:, q, mre:mre + 128],
                        sks[(d, q)][:, BT * h:BT * (h + 1)],
                        start=(q == 0), stop=(q == 3))
                for q in range(4):
                    nc.tensor.matmul(
                        frim[:], dftt[:, q, mim:mim + 128],
                        sks[(d, q)][:, BT * h:BT * (h + 1)],
                        start=(q == 0), stop=(q == 3))
                pre = prod[:, pi, :]
                pim = prod[:, 2 + pi, :]
                if d == 0:
                    # cast PSUM f32 -> fp16 directly into prod (ACT engine;
                    # GPSIMD cannot read PSUM)
                    nc.scalar.copy(pre, frre[:])
                    nc.scalar.copy(pim, frim[:])
                    if pi == 0:
                        dc = prow.tile([1, BT], F16, tag=f"dc{h}")
                        ny = prow.tile([1, BT], F16, tag=f"ny{h}")
                        nc.scalar.copy(dc[:], frre[0:1, :])
                        nc.scalar.copy(ny[:], frim[0:1, :])
                        rowfix[h] = (dc, ny)
                else:
                    fre = pfr.tile([128, BT], F16, tag="fre")
                    fim = pfr.tile([128, BT], F16, tag="fim")
                    nc.scalar.copy(fre[:], frre[:])
                    nc.scalar.copy(fim[:], frim[:])
                    if pi == 0:
                        dc0, ny0 = rowfix[h]
                        dc = prow.tile([1, BT], F16, tag=f"dc{h}")
                        ny = prow.tile([1, BT], F16, tag=f"ny{h}")
                        nc.vector.tensor_mul(dc[:], dc0[:], fre[0:1, :])
                        nc.vector.tensor_mul(ny[:], ny0[:], fim[0:1, :])
                        rowfix[h] = (dc, ny)
                    # all on DVE: GPSIMD is ~4x slower per op and was the
                    # critical path; 48 fp16 ops here are ~18us on DVE total
                    t1 = ptmp.tile([128, BT], F16, tag="t1")
                    t2 = ptmp.tile([128, BT], F16, tag="t2")
                    t3 = ptmp.tile([128, BT], F16, tag="t3")
                    t4 = ptmp.tile([128, BT], F16, tag="t4")
                    nc.vector.tensor_mul(t1[:], pre, fre[:])
                    nc.vector.tensor_mul(t2[:], pim, fim[:])
                    nc.vector.tensor_mul(t3[:], pre, fim[:])
                    nc.vector.tensor_mul(t4[:], pim, fre[:])
                    nc.vector.tensor_sub(pre, t1[:], t2[:])
                    nc.vector.tensor_add(pim, t3[:], t4[:])
                    if d == DEG - 1 and pi == 0:
                        dc, ny = rowfix[h]
                        nc.vector.tensor_copy(prod[0:1, 0, :], dc[:])
                        nc.vector.tensor_copy(prod[0:1, 2, :], ny[:])

            # ---- stage 4: irfft as matmul, drain via SBUF f32, DMA out.
            def stage4(h):
                prod = prods[h]
                for m in range(4):
                    po = ps_sk.tile([128, BT], F32, tag=f"skh{m % 2}")
                    for q in range(4):
                        nc.tensor.matmul(
                            po[:], icot[:, q, 128 * m:128 * (m + 1)],
                            prod[:, q, :],
                            start=(q == 0), stop=(q == 3))
                    ob = pout.tile([128, BT], F32, tag=f"ob{m % 2}")
                    if m % 2 == 0:
                        nc.scalar.copy(ob[:], po[:])
                    else:
                        nc.vector.tensor_copy(ob[:], po[:])
                    nc.scalar.dma_start(
                        ot[128 * m:128 * (m + 1), BT * h:BT * (h + 1)], ob[:])

            # ---- emission schedule: keep the PE queue saturated; spread
            # degree-0/1 DFTs into stage-1 so the DVE product chains overlap
            # stage-1/2 matmuls, leaving only degree-2's chain in the tail.
            di = 0
            for g in range(NBLK):
                stage1(0, g, di); di += 1
            for g in range(NBLK):
                stage1(1, g, di); di += 1
            for pi in range(2):
                stage23(0, 0, pi)
            stage1(2, 0, di); di += 1
            stage1(2, 1, di); di += 1
            for pi in range(2):
                stage23(0, 1, pi)
            stage1(2, 2, di); di += 1
            stage1(2, 3, di); di += 1
            for pi in range(2):
                stage23(1, 0, pi)
            for pi in range(2):
                stage23(1, 1, pi)
            for pi in range(2):
                stage23(2, 0, pi)
            for pi in range(2):
                stage23(2, 1, pi)
            stage4(0)
            stage4(1)

    nc.compile()
    return nc


def round_fp16(x):
    return np.asarray(x, np.float32).astype(np.float16)


def prepare_inputs(X, index_hash, bit_hash):
    order, chunks, plan, zm_t = build_plan(index_hash, bit_hash)
    dft_t, ico_t = build_dft_tables()
    nmm = zm_t.shape[1]
    # padded layout: chunk c occupies rows [128c, 128c+fill), rest zero
    Xt = np.asarray(X, np.float32).T[order]
    Xp = np.zeros((len(chunks) * 128, Xt.shape[1]), np.float16)
    for c, (start, fill) in enumerate(chunks):
        Xp[128 * c:128 * c + fill] = Xt[start:start + fill]
    shared = {
        "zm": round_fp16(zm_t),          # +-1/0: exact in fp16
        "dft": round_fp16(dft_t),
        "ico": round_fp16(ico_t),
    }
    return plan, chunks, nmm, Xp, shared


def kernel(X, index_hash, bit_hash, _trace=False):
    plan, chunks, nmm, Xp, shared = prepare_inputs(X, index_hash, bit_hash)
    nc = build_program(plan, chunks, nmm)
    in_maps = [
        {"xp": np.ascontiguousarray(Xp[:, c * B_CORE:(c + 1) * B_CORE]), **shared}
        for c in range(NCORES)
    ]
    res = bass_utils.run_bass_kernel_spmd(
        nc, in_maps, core_ids=list(range(NCORES)), trace=_trace)
    out = np.empty((B, NCOMP), np.float32)
    for c in range(NCORES):
        out[c * B_CORE:(c + 1) * B_CORE] = res.results[c]["ot"].T
    return (out, res) if _trace else out
